# revision 19
# baseline (speedup 1.0000x reference)
"""Trainium2 Bass kernel for nn_BinarizedLayer.

reference:
    upper = max(c1, c2); lower = min(c1, c2); middle = upper - lower
    w = where(weights < middle, lower, upper)
    out = input_ @ w.T + bias            # input_ [4, 4096, 1024], w [4096, 1024]

Strategy: data-parallel over the 16384 tokens across 8 NeuronCores
(2048 tokens/core). Each core computes its out-shard [2048, 4096] with
K=1024 on the SBUF partition dim, using the identity

    w_bin = lower + middle * (w >= middle)        (middle = upper - lower)
    out   = middle * (x @ mask.T) + lower * rowsum(x) + bias

so the PE matmul only ever sees the exact {0,1} mask (exact at ANY PE
precision) against x in float32r (full bf16-rate streaming for
free-dim >= 256). The mask compare runs on the DVE in full fp32, and
middle/lower*rowsum/bias are folded in by ACT+DVE during PSUM evict,
all in fp32 - so the only precision loss is the PE's float32r
truncation of x itself.
"""

import sys

for _p in ("/opt/trn_rl_repo", "/root/.axon_site/_ro/trn_rl_repo"):
    if _p not in sys.path:
        sys.path.insert(0, _p)

import numpy as np

import concourse.bacc as bacc
import concourse.mybir as mybir
import concourse.tile as tile
from concourse.bass_utils import run_bass_kernel_spmd

P = 128
B, S, DIN, DOUT = 4, 4096, 1024, 4096
NCORES = 8
TOK = B * S                # 16384 tokens
M = TOK // NCORES          # 2048 tokens per core
K = DIN                    # 1024
N = DOUT                   # 4096
KT = K // P                # 8 k-tiles
MT = M // P                # 16 m-tiles
NF = 512                   # matmul free dim (max for 4-byte moving operand)
NT = N // NF               # 8 n-slices

F32 = mybir.dt.float32
F32R = mybir.dt.float32r
OP = mybir.AluOpType


def build_nc():
    nc = bacc.Bacc(
        "TRN2",
        target_bir_lowering=False,
        debug=False,
        enable_asserts=False,
        num_devices=NCORES,
    )

    xT_d = nc.dram_tensor("xT", [K, M], F32R, kind="ExternalInput").ap()
    wT_d = nc.dram_tensor("wT", [K, N], F32, kind="ExternalInput").ap()
    bias_d = nc.dram_tensor("bias", [N], F32, kind="ExternalInput").ap()
    c1_d = nc.dram_tensor("c1", [1], F32, kind="ExternalInput").ap()
    c2_d = nc.dram_tensor("c2", [1], F32, kind="ExternalInput").ap()
    ones_d = nc.dram_tensor("ones", [P, 1], F32R, kind="ExternalInput").ap()
    out_d = nc.dram_tensor("out", [M, N], F32, kind="ExternalOutput").ap()

    xT_v = xT_d.rearrange("(ko p) m -> p ko m", p=P)
    wT_v = wT_d.rearrange("(ko p) n -> p ko n", p=P)
    out_v = out_d.rearrange("(mo p) n -> p mo n", p=P)

    with tile.TileContext(nc) as tc:
        with (
            tc.tile_pool(name="const", bufs=1) as const,
            tc.tile_pool(name="xres", bufs=1) as xres,
            tc.tile_pool(name="wpool", bufs=3) as wpool,
            tc.tile_pool(name="mpool", bufs=2) as mpool,
            tc.tile_pool(name="opool", bufs=6) as opool,
            tc.tile_pool(name="pspool", bufs=7, space="PSUM") as pspool,
            tc.tile_pool(name="rspool", bufs=1, space="PSUM") as rspool,
            tc.tile_pool(name="dram", bufs=1, space="DRAM") as dram,
        ):
            # Runtime scalars replicated to all partitions: lower / middle
            c1_t = const.tile([P, 1], F32)
            c2_t = const.tile([P, 1], F32)
            nc.sync.dma_start(c1_t[:], c1_d.to_broadcast((P, 1)))
            nc.sync.dma_start(c2_t[:], c2_d.to_broadcast((P, 1)))
            lower_t = const.tile([P, 1], F32)
            middle_t = const.tile([P, 1], F32)
            upper_t = const.tile([P, 1], F32)
            nc.vector.tensor_tensor(upper_t[:], c1_t[:], c2_t[:], OP.max)
            nc.vector.tensor_tensor(lower_t[:], c1_t[:], c2_t[:], OP.min)
            nc.vector.tensor_tensor(middle_t[:], upper_t[:], lower_t[:], OP.subtract)

            # bias replicated across partitions so the evict-add reads it per-partition
            bias_t = const.tile([P, N], F32)
            nc.sync.dma_start(bias_t[:], bias_d[None, :].to_broadcast((P, N)))

            # ones column for the rowsum matmuls
            ones_t = const.tile([P, 1], F32R)
            nc.sync.dma_start(ones_t[:], ones_d)

            # Resident x^T [P, KT, M] (8 MiB). The first chunk is split into
            # k-halves so the first matmuls can start sooner; chunks 1-3 are
            # emitted inside the nt=0 m-loop so they stream behind compute.
            xT_sb = xres.tile([P, KT, M], F32R)
            MCH = M // 4
            for kp in range(0, KT, 2):
                nc.sync.dma_start(
                    xT_sb[:, kp : kp + 2, 0:MCH], xT_v[:, kp : kp + 2, 0:MCH]
                )

            # lower*rowsum(x) per token, tokens on partitions: rs_all[:, mt].
            # rowsum rows are computed on the PE (ones^T . xT per 512-token
            # chunk), then transposed into per-partition form via a DRAM
            # bounce; filled in inside the nt=0 m-loop below.
            rs_all = const.tile([P, MT], F32)
            rs_row = const.tile([1, M], F32)
            rs_dram = dram.tile([M], F32)
            rs_dram_v = rs_dram.rearrange("(mo p) -> p mo", p=P)

            # produce the exact {0,1} mask for slice nt: DMA the fp32 weights
            # and compare in full fp32 on the DVE, writing an f32r tile. The
            # first slice is split into k-pairs so the first matmuls can start
            # while the rest streams in.
            masks = {}

            def emit_mask(nt):
                w_t = wpool.tile([P, KT, NF], F32)
                m_t = mpool.tile([P, KT, NF], F32R)
                sl = slice(nt * NF, (nt + 1) * NF)
                if nt == 0:
                    for kp in range(0, KT, 2):
                        nc.sync.dma_start(
                            w_t[:, kp : kp + 2], wT_v[:, kp : kp + 2, sl]
                        )
                        nc.vector.tensor_scalar(
                            m_t[:, kp : kp + 2],
                            w_t[:, kp : kp + 2],
                            middle_t[:],
                            None,
                            OP.is_ge,
                        )
                else:
                    nc.sync.dma_start(w_t[:], wT_v[:, :, sl])
                    nc.vector.tensor_scalar(m_t[:], w_t[:], middle_t[:], None, OP.is_ge)
                masks[nt] = m_t

            emit_mask(0)
            for nt in range(NT):
                m_t = masks.pop(nt)
                for mt in range(MT):
                    # produce the next slice's mask early: it lands ahead of
                    # most of this slice's evict-ADDs in the DVE FIFO
                    if mt == 1 and nt + 1 < NT:
                        emit_mask(nt + 1)
                    if nt == 0:
                        # prefetch the next resident-x^T chunk
                        if mt % 4 == 0 and mt < 12:
                            i = mt // 4 + 1
                            nc.sync.dma_start(
                                xT_sb[:, :, i * MCH : (i + 1) * MCH],
                                xT_v[:, :, i * MCH : (i + 1) * MCH],
                            )
                        # rowsum for this 512-token chunk (feeds the next four
                        # epilogues): PE: rs_ps[1, 512] = ones^T @ xT, then a
                        # DRAM bounce to per-partition rs_all[:, 4i:4i+4],
                        # scaled by `lower`.
                        if mt % 4 == 0:
                            i = mt // 4
                            sl = slice(i * MCH, (i + 1) * MCH)
                            rs_ps = rspool.tile([1, MCH], F32)
                            for kt in range(KT):
                                nc.tensor.matmul(
                                    rs_ps[:],
                                    ones_t[:],
                                    xT_sb[:, kt, sl],
                                    start=(kt == 0),
                                    stop=(kt == KT - 1),
                                )
                            nc.vector.tensor_copy(rs_row[:, sl], rs_ps[:])
                            nc.sync.dma_start(rs_dram[sl], rs_row[:, sl])
                            nc.sync.dma_start(
                                rs_all[:, i * 4 : (i + 1) * 4],
                                rs_dram_v[:, i * 4 : (i + 1) * 4],
                            )
                            nc.vector.tensor_scalar(
                                rs_all[:, i * 4 : (i + 1) * 4],
                                rs_all[:, i * 4 : (i + 1) * 4],
                                lower_t[:],
                                None,
                                OP.mult,
                            )
                    ps = pspool.tile([P, NF], F32)
                    for kt in range(KT):
                        nc.tensor.matmul(
                            ps[:],
                            xT_sb[:, kt, mt * P : (mt + 1) * P],
                            m_t[:, kt, :],
                            start=(kt == 0),
                            stop=(kt == KT - 1),
                        )
                    o_t = opool.tile([P, NF], F32)
                    # ACT: o = middle * psum + lower*rowsum[m]  (both per-partition APs)
                    nc.scalar.activation(
                        o_t[:],
                        ps[:],
                        mybir.ActivationFunctionType.Identity,
                        bias=rs_all[:, mt : mt + 1],
                        scale=middle_t[:],
                    )
                    # DVE: o += bias[n]
                    nc.vector.tensor_tensor(
                        o_t[:], o_t[:], bias_t[:, nt * NF : (nt + 1) * NF], OP.add
                    )
                    nc.sync.dma_start(out_v[:, mt, nt * NF : (nt + 1) * NF], o_t[:])

    nc.compile()
    return nc


_NC_CACHE = None


def _get_nc():
    global _NC_CACHE
    if _NC_CACHE is None:
        _NC_CACHE = build_nc()
    return _NC_CACHE


def make_in_maps(input_, weights, c1, c2, bias):
    x = np.ascontiguousarray(np.asarray(input_, dtype=np.float32)).reshape(TOK, DIN)
    wT = np.ascontiguousarray(np.asarray(weights, dtype=np.float32).T)
    bias = np.ascontiguousarray(np.asarray(bias, dtype=np.float32))
    c1 = np.ascontiguousarray(np.asarray(c1, dtype=np.float32))
    c2 = np.ascontiguousarray(np.asarray(c2, dtype=np.float32))
    ones = np.ones((P, 1), dtype=np.float32)
    in_maps = []
    for c in range(NCORES):
        xT_c = np.ascontiguousarray(x[c * M : (c + 1) * M].T)
        in_maps.append(
            {"xT": xT_c, "wT": wT, "bias": bias, "c1": c1, "c2": c2, "ones": ones}
        )
    return in_maps


def run(in_maps, trace=False, **kwargs):
    return run_bass_kernel_spmd(
        _get_nc(), in_maps, core_ids=list(range(NCORES)), trace=trace, **kwargs
    )


def kernel(input_, weights, c1, c2, bias):
    in_maps = make_in_maps(input_, weights, c1, c2, bias)
    res = run(in_maps, trace=False)
    out = np.concatenate([r["out"] for r in res.results], axis=0)
    return out.reshape(B, S, DOUT).astype(np.float32)
